# revision 27
# baseline (speedup 1.0000x reference)
"""CIGLoss (segment_reduce) Trainium2 kernel.

Strategy (data-parallel over batch, per the sharding hint):
  - Each of the 8 NeuronCores owns one image and that image's pixel list
    (segments are image-local: seg // 500 == image).
  - Host-side sharding packs each image's ~500 segments into a
    [128 partitions, NSLOT slots, L] padded grid (one whole segment per
    slot).  Pad entries point at a zero element appended to the image, so
    they contribute 0 to every sum.
  - The value lookup input[b,0,row,col] happens during host packing (this
    toolchain's walrus mis-lowers per-element indirect DMA: one descriptor
    per contiguous dest run, only the run-start offset honored — verified
    by hardware probes; see hw_gather_probe*.py).  All reductions run on
    device, per-slot:
        sums  = reduce_add(v)            counts = reduce_add(row < H)
        mean  = sums * recip(max(counts,1))
        dev   = reduce_add(|v - mean|)   contrib = dev * recip
    and a final partition reduce to one scalar per core.
  - Host sums the 8 per-core partials and divides by B.
"""

import numpy as np

_NUM_PATHS = 4000
_P = 128  # SBUF partitions


def _build_nc(nslot: int, L: int, ntot: int, W: int, H: int, chunk: int):
    import concourse.bacc as bacc
    import concourse.bass as bass
    import concourse.tile as tile
    from concourse import mybir

    f32 = mybir.dt.float32
    i32 = mybir.dt.int32
    Alu = mybir.AluOpType
    Ax = mybir.AxisListType
    FREE = nslot * L

    assert L % chunk == 0 or chunk % L == 0
    nch = FREE // chunk
    spc = max(1, chunk // L)   # whole slots per chunk (when chunk >= L)
    cps = max(1, L // chunk)   # chunks per slot (when chunk < L)

    nc = bacc.Bacc("TRN2", debug=False)
    v_d = nc.dram_tensor("vP", [_P, FREE], f32, kind="ExternalInput")
    rows_d = nc.dram_tensor("rowsP", [_P, FREE], i32, kind="ExternalInput")
    out_d = nc.dram_tensor("out", [_P, 1], f32, kind="ExternalOutput")

    _emit(nc, tile, bass, nslot, L, W, H, chunk, f32, i32, Alu, Ax,
          v_d, rows_d, out_d, FREE, nch, spc, cps)
    # Bacc defers register allocation + wait-splitting to finalize(); the
    # pjrt run path serializes the module as-is, so finalize here.
    nc.finalize()
    return nc


def _emit(nc, tile, bass, nslot, L, W, H, chunk, f32, i32, Alu, Ax,
          v_d, rows_d, out_d, FREE, nch, spc, cps):
    with tile.TileContext(nc) as tc:
        with (
            tc.tile_pool(name="big", bufs=1) as big,
            tc.tile_pool(name="small", bufs=1) as small,
        ):
            rows_t = big.tile([_P, FREE], i32)
            nc.sync.dma_start(out=rows_t[:], in_=rows_d[:, :])

            # indicator of real (non-pad) pixels: row < H
            ind_t = big.tile([_P, FREE], f32)
            nc.vector.tensor_scalar(
                out=ind_t[:], in0=rows_t[:], scalar1=H, scalar2=None,
                op0=Alu.is_lt,
            )

            # gathered pixel values in slot layout; chunked load with
            # per-chunk partial sums so load and reduce overlap.
            v_t = big.tile([_P, FREE], f32)
            psum_t = small.tile([_P, nch * spc], f32)
            for k in range(nch):
                a, b = k * chunk, (k + 1) * chunk
                nc.sync.dma_start(out=v_t[:, a:b], in_=v_d[:, a:b])
                nc.vector.tensor_reduce(
                    out=psum_t[:, k * spc:(k + 1) * spc],
                    in_=v_t[:, a:b].rearrange("p (s l) -> p s l", s=spc),
                    axis=Ax.X, op=Alu.add,
                )

            v3 = v_t[:].rearrange("p (s l) -> p s l", s=nslot)
            ind3 = ind_t[:].rearrange("p (s l) -> p s l", s=nslot)

            # combine per-chunk partials into per-slot sums
            sums = small.tile([_P, nslot], f32)
            if cps == 1:
                nc.vector.tensor_copy(out=sums[:], in_=psum_t[:])
            elif cps == 2:
                nc.vector.tensor_tensor(
                    out=sums[:], in0=psum_t[:, 0::2], in1=psum_t[:, 1::2],
                    op=Alu.add,
                )
            else:
                nc.vector.tensor_reduce(
                    out=sums[:],
                    in_=psum_t[:].rearrange("p (s c) -> p s c", s=nslot),
                    axis=Ax.X, op=Alu.add,
                )
            counts = small.tile([_P, nslot], f32)
            nc.vector.tensor_reduce(out=counts[:], in_=ind3, axis=Ax.X, op=Alu.add)
            nc.vector.tensor_scalar_max(counts[:], counts[:], 1.0)
            w_t = small.tile([_P, nslot], f32)
            nc.vector.reciprocal(w_t[:], counts[:])
            means = small.tile([_P, nslot], f32)
            nc.vector.tensor_tensor(
                out=means[:], in0=sums[:], in1=w_t[:], op=Alu.mult
            )

            x_t = big.tile([_P, FREE], f32)
            x3 = x_t[:].rearrange("p (s l) -> p s l", s=nslot)
            nc.vector.tensor_tensor(
                out=x3, in0=v3, in1=means[:].to_broadcast([_P, nslot, L]),
                op=Alu.subtract,
            )
            devs = small.tile([_P, nslot], f32)
            nc.vector.tensor_reduce(
                out=devs[:], in_=x3, axis=Ax.X, op=Alu.add,
                apply_absolute_value=True,
            )
            # pads were gathered as 0, so each contributed |0 - mean| to devs;
            # subtract the known pad contribution (L - count) * |mean|.
            npad = small.tile([_P, nslot], f32)
            nc.vector.tensor_scalar(
                out=npad[:], in0=counts[:], scalar1=-1.0, scalar2=float(L),
                op0=Alu.mult, op1=Alu.add,
            )
            absm = small.tile([_P, nslot], f32)
            nc.vector.tensor_scalar(
                out=absm[:], in0=means[:], scalar1=-1.0, scalar2=None, op0=Alu.mult
            )
            nc.vector.tensor_tensor(
                out=absm[:], in0=absm[:], in1=means[:], op=Alu.max
            )
            nc.vector.tensor_tensor(
                out=npad[:], in0=npad[:], in1=absm[:], op=Alu.mult
            )
            nc.vector.tensor_tensor(
                out=devs[:], in0=devs[:], in1=npad[:], op=Alu.subtract
            )
            contrib = small.tile([_P, nslot], f32)
            nc.vector.tensor_tensor(
                out=contrib[:], in0=devs[:], in1=w_t[:], op=Alu.mult
            )
            part = small.tile([_P, 1], f32)
            nc.vector.tensor_reduce(
                out=part[:], in_=contrib[:], axis=Ax.X, op=Alu.add
            )
            nc.sync.dma_start(out=out_d[:, :], in_=part[:])
    return nc


_CACHE = {}


def _get_nc(key):
    if key not in _CACHE:
        _CACHE[key] = _build_nc(*key)
    return _CACHE[key]


def _pack(input, rows, cols, seg_ids, num_paths):
    """Host-side sharding: one image per core, segments packed into a
    [ncore, 128, nslot*L] padded slot grid."""
    B, C, H, W = input.shape
    ppi = num_paths // B  # paths (segments) per image
    npix = rows.shape[0]

    bnd = np.searchsorted(seg_ids, np.arange(num_paths + 1)).astype(np.int64)
    seg_lens = np.diff(bnd)
    nslot = int(np.ceil(ppi / _P))
    lmax = int(seg_lens.max()) if npix else 1
    L = max(128, int(np.ceil(lmax / 128.0)) * 128)
    FREE = nslot * L

    s = np.arange(num_paths)
    core = s // ppi
    local = s % ppi
    part = local % _P
    slot = local // _P
    base = ((core * _P + part) * np.int64(nslot) + slot) * L
    dest = np.repeat(base, seg_lens) + (
        np.arange(npix, dtype=np.int64) - np.repeat(bnd[:-1], seg_lens)
    )
    rows_p = np.full(B * _P * FREE, H, np.int32)
    rows_p[dest] = rows
    # Pixel values in slot layout.  This lookup runs on the host: the
    # toolchain's walrus build mis-lowers sub-row indirect DMA (one
    # descriptor per contiguous dest run, only the run-start offset is
    # honored), so a per-element device gather is not expressible; all
    # reductions stay on device.
    core_of = np.repeat(core, seg_lens)
    v_p = np.zeros(B * _P * FREE, np.float32)
    v_p[dest] = input[core_of, 0, rows, cols]
    return (v_p.reshape(B, _P, FREE), rows_p.reshape(B, _P, FREE),
            nslot, L, H * W + 128)


def kernel(input, rows, cols, seg_ids, _trace=False, _num_paths=_NUM_PATHS):
    from concourse.bass_utils import run_bass_kernel_spmd

    input = np.ascontiguousarray(np.asarray(input, np.float32))
    rows = np.ascontiguousarray(np.asarray(rows, np.int32))
    cols = np.ascontiguousarray(np.asarray(cols, np.int32))
    seg_ids = np.ascontiguousarray(np.asarray(seg_ids, np.int32))
    B, C, H, W = input.shape

    v_p, rows_p, nslot, L, ntot = _pack(input, rows, cols, seg_ids, _num_paths)
    chunk = L // 2 if (L % 2 == 0 and L >= 512) else L
    nc = _get_nc((nslot, L, ntot, W, H, chunk))
    in_maps = [
        {"vP": v_p[i], "rowsP": rows_p[i]} for i in range(B)
    ]
    res = run_bass_kernel_spmd(nc, in_maps, core_ids=list(range(B)), trace=_trace)
    total = sum(float(r["out"].sum()) for r in res.results)
    out = np.float32(total / B)
    if _trace:
        return out, res
    return out
